# revision 39
# baseline (speedup 1.0000x reference)
"""Adaptive Computation Time step kernel for Trainium2 (8 NeuronCores).

Pure data parallel over batch: each of the 8 cores handles 4 of the 32
batch rows. Token layout is c-major: token j of a batch row lives at SBUF
slot (partition j%128, chunk j//128), matching dma_gather/dma_scatter_add
sequence placement (seq i -> partition i%128, block i//128).

Per batch row (M=2048 tokens, H=1024):
  1. running-mask cumsum in c-major: two triangular matmuls on PE
     (per-column inclusive + replicated column sums) plus a free-dim
     scan for the cross-column prefixes
  2. unpack gather indices rewrapped to the 16-partition-wrapped
     dma_gather layout via PE transposes (1 big + 8 small [16,16]) and a
     replication matmul; dma_gather of 4KB h rows into SBUF (4 x 512 rows)
  3. q = h_un . W as a fused scalar_tensor_tensor with accum_out on DVE
     (one op per 1024-wide chunk); sigmoid(+bias) on ACT; halting-mask
     algebra on DVE with padj = runf*(1-acc) + cont*(acc_new-1)
  4. acc_new transposed back via PE and stored contiguously
  5. weighted_new = h*padj + wh*(1-padj): h*padj on ACT/Pool (per-
     partition-scalar multiplies), the rest fused on DVE; wh loads on the
     SP HWDGE ring, weighted_new stores on the ACT HWDGE ring
  6. pack: rank_new cumsum + scatter indices (dropped tokens target row
     bi*M+M-1), dma_scatter_add into the pre-zeroed packed output, then
     row bi*M+M-1 is rewritten with zeros (ordered via add_dep_helper)

Known toolchain constraints baked in: tensor_tensor_reduce and Pool-side
scalar_tensor_tensor are rejected by this runtime/compiler; dma_gather/
dma_scatter_add need single_packet=False and num_idxs <= 1024 per call;
gpsimd.load_library must NOT be called (the NEFF handles ucode loading).
"""

import sys

import numpy as np

B, M, H = 32, 2048, 1024
NCORES = 8
BL = B // NCORES          # batch rows per core
M4 = BL * M               # rows of h per core
P, C = 128, 16            # token j = c*128 + p (c-major)
HC = 4                    # chunks per weighted-stream group
NG = C // HC
THRESH = 0.99

_cache = {}


def _ensure_path():
    if "/opt/trn_rl_repo" not in sys.path:
        sys.path.insert(0, "/opt/trn_rl_repo")


def build_nc():
    _ensure_path()
    import concourse.bass as bass
    import concourse.bacc as bacc
    import concourse.mybir as mybir
    from concourse.bass import _add_dep_helper
    from concourse.tile import TileContext
    f32 = mybir.dt.float32
    i16 = mybir.dt.int16
    u8 = mybir.dt.uint8
    Alu = mybir.AluOpType
    Act = mybir.ActivationFunctionType

    nc = bacc.Bacc(None, target_bir_lowering=False)
    hP = nc.declare_dram_parameter("h", [M4, H], f32, isOutput=False)
    wP = nc.declare_dram_parameter("W", [1, H], f32, isOutput=False)
    bP = nc.declare_dram_parameter("bvec", [1, 1], f32, isOutput=False)
    whP = nc.declare_dram_parameter("wh", [M4, H], f32, isOutput=False)
    accP = nc.declare_dram_parameter("acc", [BL, M], f32, isOutput=False)
    runP = nc.declare_dram_parameter("run", [BL, M], u8, isOutput=False)
    uiP = nc.declare_dram_parameter("ui", [P, P], f32, isOutput=False)
    onP = nc.declare_dram_parameter("ones", [P, P], f32, isOutput=False)
    repP = nc.declare_dram_parameter("rep", [P, P], f32, isOutput=False)
    idnP = nc.declare_dram_parameter("idn", [P, P], f32, isOutput=False)
    pkO = nc.declare_dram_parameter("packed", [M4, H], f32, isOutput=True)
    wnO = nc.declare_dram_parameter("wnew", [M4, H], f32, isOutput=True)
    anO = nc.declare_dram_parameter("accn", [BL, M], f32, isOutput=True)

    with TileContext(nc) as tc:
        with (
            tc.tile_pool(name="const", bufs=1) as cpool,
            tc.tile_pool(name="big", bufs=2) as bigp,
            tc.tile_pool(name="whp", bufs=2) as whp,
            tc.tile_pool(name="scr", bufs=2) as scrp,
            tc.tile_pool(name="small", bufs=3) as smp,
            tc.tile_pool(name="wrap", bufs=3) as wrp,
            tc.tile_pool(name="psum", bufs=1, space="PSUM") as psp,
        ):
            ui_t = cpool.tile([P, P], f32)
            nc.sync.dma_start(ui_t[:], uiP[:])
            on_t = cpool.tile([P, P], f32)
            nc.sync.dma_start(on_t[:], onP[:])
            rep_t = cpool.tile([P, P], f32)
            nc.sync.dma_start(rep_t[:], repP[:])
            idn_t = cpool.tile([P, P], f32)
            nc.sync.dma_start(idn_t[:], idnP[:])
            Wb = cpool.tile([P, H], f32)
            nc.sync.dma_start(Wb[:], wP[:].to_broadcast((P, H)))
            bias = cpool.tile([P, 1], f32)
            nc.sync.dma_start(bias[:], bP[:].to_broadcast((P, 1)))
            zrow = cpool.tile([1, H], f32)
            nc.vector.memset(zrow[:], 0.0)
            actwarm = cpool.tile([1, 1], f32)
            nc.vector.memset(actwarm[:], 0.0)
            nc.scalar.activation(actwarm[:], actwarm[:], Act.Sigmoid)

            h_all = hP[:]                                       # [M4, H]
            pk_all = pkO[:]                                     # [M4, H]
            acc_r = accP[:].rearrange("b (r f) -> b r f", r=16)   # [BL,16,128]
            run_r = runP[:].rearrange("b (r f) -> b r f", r=16)
            an_r = anO[:].rearrange("b (r f) -> b r f", r=16)
            wh_r = whP[:].rearrange("(b c p) e -> b p c e", b=BL, c=C)
            wn_r = wnO[:].rearrange("(b c p) e -> b p c e", b=BL, c=C)

            def cumsum_cm(x_cm, tagpfx):
                """Inclusive cumsum over j=c*128+p of a [128,16] c-major tile."""
                b1 = psp.tile([P, C], f32, tag="ps_b1")
                nc.tensor.matmul(b1[:], lhsT=ui_t[:], rhs=x_cm[:],
                                 start=True, stop=True)
                cs = psp.tile([P, C], f32, tag="ps_cs")
                nc.tensor.matmul(cs[:], lhsT=on_t[:], rhs=x_cm[:],
                                 start=True, stop=True)
                incl = smp.tile([P, C], f32, tag=tagpfx + "_incl")
                nc.vector.tensor_tensor_scan(
                    incl[:], cs[:], x_cm[:], 0.0, Alu.add, Alu.bypass
                )
                excl = smp.tile([P, C], f32, tag=tagpfx + "_excl")
                nc.vector.tensor_tensor(excl[:], incl[:], cs[:], Alu.subtract)
                cum = smp.tile([P, C], f32, tag=tagpfx + "_cum")
                nc.vector.tensor_tensor(cum[:], excl[:], b1[:], Alu.add)
                return cum

            def wrap_idx(x_cm, tagpfx):
                """[128,16] c-major f32 -> [128,128] int16 16-wrapped,
                replicated across the 8 partition groups."""
                t1 = psp.tile([16, P], f32, tag="ps_tbig", bufs=2)
                nc.tensor.transpose(out=t1[:], in_=x_cm[:], identity=idn_t[:])
                a1 = wrp.tile([16, P], f32, tag=tagpfx + "_a1")
                nc.vector.tensor_copy(a1[:], t1[:])
                w16 = wrp.tile([16, P], f32, tag=tagpfx + "_w16")
                for w in range(8):
                    vw = psp.tile([16, 16], f32, tag="ps_tsm", bufs=2)
                    nc.tensor.transpose(
                        out=vw[:], in_=a1[:, 16 * w : 16 * (w + 1)],
                        identity=idn_t[0:16, 0:16],
                    )
                    nc.vector.tensor_copy(w16[:, w::8], vw[:])
                rw = psp.tile([P, P], f32, tag="ps_rw")
                nc.tensor.matmul(rw[:], lhsT=rep_t[0:16, :], rhs=w16[:],
                                 start=True, stop=True)
                idx = wrp.tile([P, P], i16, tag=tagpfx + "_idx")
                nc.vector.tensor_copy(idx[:], rw[:])
                return idx

            def load_cm(dram_ap, dtype, tagpfx):
                """Contiguous [16,128] load + PE transpose -> [128,16] c-major."""
                t16 = smp.tile([16, P], dtype, tag=tagpfx + "_t16")
                nc.sync.dma_start(t16[:], dram_ap)
                if dtype != f32:
                    tf = smp.tile([16, P], f32, tag=tagpfx + "_tf")
                    nc.vector.tensor_copy(tf[:], t16[:])
                    t16 = tf
                ps = psp.tile([P, C], f32, tag="ps_tin")
                nc.tensor.transpose(out=ps[:], in_=t16[:],
                                    identity=idn_t[0:16, 0:16])
                cm = smp.tile([P, C], f32, tag=tagpfx + "_cm")
                nc.vector.tensor_copy(cm[:], ps[:])
                return cm

            def run_batch(bi):
                runf = load_cm(run_r[bi], u8, "run")
                acc = load_cm(acc_r[bi], f32, "acc")

                # ---- unpack gather indices ----
                cum1 = cumsum_cm(runf, "c1")
                idxf = smp.tile([P, C], f32, tag="idxf")
                nc.vector.tensor_scalar(
                    idxf[:], cum1[:], scalar1=float(bi * M - 1),
                    scalar2=float(bi * M), op0=Alu.add, op1=Alu.max,
                )
                gidx = wrap_idx(idxf, "g")

                # ---- gather h rows ----
                hrows = bigp.tile([P, C * H], f32, tag="hrows")
                hrows3 = hrows[:].rearrange("p (c e) -> p c e", e=H)
                for k in range(4):
                    nc.gpsimd.dma_gather(
                        hrows3[:, 4 * k : 4 * (k + 1), :],
                        h_all,
                        gidx[:, 32 * k : 32 * (k + 1)],
                        M // 4,
                        M // 4,
                        H,
                        single_packet=False,
                    )
                # ---- q = h_un . W ----
                q = smp.tile([P, C], f32, tag="q")
                for c in range(C):
                    scr = scrp.tile([P, H], f32, tag="ttr_scr")
                    nc.vector.scalar_tensor_tensor(
                        scr[:],
                        in0=hrows[:, c * H : (c + 1) * H],
                        scalar=1.0,
                        in1=Wb[:],
                        op0=Alu.mult,
                        op1=Alu.mult,
                        accum_out=q[:, c : c + 1],
                    )

                # ---- halting algebra (c-major [128,16]) ----
                sig = smp.tile([P, C], f32, tag="sig")
                nc.scalar.activation(
                    sig[:], q[:], Act.Sigmoid, bias=bias[:, 0:1], scale=1.0
                )
                pt = smp.tile([P, C], f32, tag="pt")
                nc.vector.tensor_tensor(pt[:], sig[:], runf[:], Alu.mult)
                accn = smp.tile([P, C], f32, tag="accn")
                nc.vector.tensor_tensor(accn[:], acc[:], pt[:], Alu.add)
                # acc_new out: transpose back to [16,128] and store contiguous
                an_ps = psp.tile([16, P], f32, tag="ps_tin")
                nc.tensor.transpose(out=an_ps[:], in_=accn[:], identity=idn_t[:])
                an_s = smp.tile([16, P], f32, tag="an_s")
                nc.vector.tensor_copy(an_s[:], an_ps[:])
                nc.sync.dma_start(an_r[bi], an_s[:])

                # cont = (acc_new < T) * runf, fused
                cont = smp.tile([P, C], f32, tag="cont")
                nc.vector.scalar_tensor_tensor(
                    cont[:], in0=accn[:], scalar=THRESH, in1=runf[:],
                    op0=Alu.is_lt, op1=Alu.mult,
                )
                # padj = runf*(1-acc) + cont*(acc_new-1)
                oma = smp.tile([P, C], f32, tag="oma")
                nc.vector.tensor_scalar(
                    oma[:], acc[:], scalar1=-1.0, scalar2=1.0,
                    op0=Alu.mult, op1=Alu.add,
                )
                t1v = smp.tile([P, C], f32, tag="t1v")
                nc.vector.tensor_tensor(t1v[:], oma[:], runf[:], Alu.mult)
                t2v = smp.tile([P, C], f32, tag="t2v")
                nc.vector.scalar_tensor_tensor(
                    t2v[:], in0=accn[:], scalar=-1.0, in1=cont[:],
                    op0=Alu.add, op1=Alu.mult,
                )
                padj = smp.tile([P, C], f32, tag="padj")
                nc.vector.tensor_tensor(padj[:], t1v[:], t2v[:], Alu.add)
                omp = smp.tile([P, C], f32, tag="omp")
                nc.vector.tensor_scalar(
                    omp[:], padj[:], scalar1=-1.0, scalar2=1.0,
                    op0=Alu.mult, op1=Alu.add,
                )

                # ---- pack scatter indices ----
                cum2 = cumsum_cm(cont, "c2")
                # kept -> cum2-1+bi*M ; dropped -> bi*M+M-1 (always a zero row
                # when any dropped token exists; its source rows are zeroed)
                v1 = smp.tile([P, C], f32, tag="v1")
                nc.vector.scalar_tensor_tensor(
                    v1[:], in0=cum2[:], scalar=-1.0, in1=cont[:],
                    op0=Alu.add, op1=Alu.mult,
                )
                tdrop = smp.tile([P, C], f32, tag="tdrop")
                nc.vector.tensor_scalar(
                    tdrop[:], cont[:], scalar1=-float(M - 1),
                    scalar2=float(bi * M + M - 1), op0=Alu.mult, op1=Alu.add,
                )
                sidxf = smp.tile([P, C], f32, tag="sidxf")
                nc.vector.tensor_tensor(sidxf[:], v1[:], tdrop[:], Alu.add)
                sidx = wrap_idx(sidxf, "s")

                # ---- weighted_new, streamed in groups of HC chunks ----
                for g in range(NG):
                    whg = whp.tile([P, HC * H], f32, tag="whg")
                    whg3 = whg[:].rearrange("p (c e) -> p c e", e=H)
                    nc.sync.dma_start(whg3, wh_r[bi, :, g * HC : (g + 1) * HC, :])
                    for cc in range(HC):
                        c = g * HC + cc
                        hp = scrp.tile([P, H], f32, tag="hp", bufs=3)
                        if cc % 2 == 0:
                            nc.scalar.mul(
                                hp[:], hrows[:, c * H : (c + 1) * H],
                                mul=padj[:, c : c + 1],
                            )
                        else:
                            nc.gpsimd.tensor_scalar(
                                hp[:], hrows[:, c * H : (c + 1) * H],
                                scalar1=padj[:, c : c + 1], scalar2=None,
                                op0=Alu.mult,
                            )
                        nc.vector.scalar_tensor_tensor(
                            whg[:, cc * H : (cc + 1) * H],
                            in0=whg[:, cc * H : (cc + 1) * H],
                            scalar=omp[:, c : c + 1],
                            in1=hp[:],
                            op0=Alu.mult,
                            op1=Alu.add,
                        )
                    nc.scalar.dma_start(wn_r[bi, :, g * HC : (g + 1) * HC, :], whg3)

                # ---- scatter pack; dropped rows dump garbage onto row
                # bi*M+M-1 (never a kept rank unless count_new==M), which is
                # then overwritten with zeros after both scatter halves ----
                scats = []
                for k in range(2):
                    scats.append(
                        nc.gpsimd.dma_scatter_add(
                            pk_all,
                            hrows3[:, 8 * k : 8 * (k + 1), :],
                            sidx[:, 64 * k : 64 * (k + 1)],
                            M // 2,
                            M // 2,
                            H,
                            single_packet=False,
                        )
                    )
                zf = nc.sync.dma_start(
                    pk_all[bi * M + M - 1 : bi * M + M, :], zrow[0:1, :]
                )
                for s in scats:
                    _add_dep_helper(
                        zf.ins, s.ins, sync=True,
                        reason="zero-fix row M-1 after scatter garbage",
                    )

            for bi in range(BL):
                run_batch(bi)

    nc.finalize()
    return nc


def _get_nc():
    if "nc" not in _cache:
        _cache["nc"] = build_nc()
    return _cache["nc"]


def _consts():
    ui = (np.arange(P)[:, None] <= np.arange(P)[None, :]).astype(np.float32)
    ones = np.ones((P, P), np.float32)
    rep = np.zeros((P, P), np.float32)
    for p in range(16):
        rep[p, p::16] = 1.0
    idn = np.eye(P, dtype=np.float32)
    return ui, ones, rep, idn


def make_in_maps(h, W, b, weighted_h, acc_p, run):
    ui, ones, rep, idn = _consts()
    in_maps = []
    for i in range(NCORES):
        sl = slice(i * BL, (i + 1) * BL)
        in_maps.append(
            {
                "h": np.ascontiguousarray(h[sl]).reshape(M4, H),
                "W": np.ascontiguousarray(W).reshape(1, H),
                "bvec": np.ascontiguousarray(b).reshape(1, 1),
                "wh": np.ascontiguousarray(weighted_h[sl]).reshape(M4, H),
                "acc": np.ascontiguousarray(acc_p[sl]).reshape(BL, M),
                "run": np.ascontiguousarray(
                    run[sl].astype(np.uint8)
                ).reshape(BL, M),
                "ui": ui,
                "ones": ones,
                "rep": rep,
                "idn": idn,
            }
        )
    return in_maps


def kernel(h, W, b, weighted_h, acc_p, run):
    _ensure_path()
    from concourse.bass_utils import run_bass_kernel_spmd

    nc = _get_nc()
    in_maps = make_in_maps(h, W, b, weighted_h, acc_p, run)
    res = run_bass_kernel_spmd(nc, in_maps, core_ids=list(range(NCORES))).results

    packed = np.concatenate(
        [r["packed"].reshape(BL, M, H) for r in res], axis=0
    ).astype(np.float32)
    wnew = np.concatenate(
        [r["wnew"].reshape(BL, M, H) for r in res], axis=0
    ).astype(np.float32)
    accn = np.concatenate(
        [r["accn"].reshape(BL, M, 1) for r in res], axis=0
    ).astype(np.float32)
    return packed, wnew, accn
